# revision 20
# baseline (speedup 1.0000x reference)
"""Multi-head attention TRN2 kernel.

Problem: B=4, S=2048, E=1024, H=16 heads of D=64.
Sharding: tensor parallelism over heads — each of the 8 cores computes 2 heads
end-to-end (QKV projections, attention, its slice of the output projection) and
returns a partial [B*S, E] output; the host sums the 8 partials and adds the
combined bias constant.

Per-core dataflow (all matmuls bf16 inputs, fp32 PSUM accumulation):
  - host passes x pre-transposed (xT: [B, E, S]) so Q^T/K^T come straight out
    of the PE in [d, s] layout and V in natural [t, d] layout — no on-chip
    transposes anywhere.
  - scores are computed transposed (S^T: [t, s]) so softmax's reduction (over
    t) can be fused into the attention*V matmul: V gets an appended column of
    ones, so row 64 of the AV output is the softmax denominator.
  - exp runs on the scalar engine with the 1/sqrt(D) scale folded in; scores
    are bounded (|s|<3) so no max-subtraction pass is needed.
  - normalization multiplies the AV output by 1/denominator broadcast across
    partitions via a K=1 matmul (ones ⊗ recip).
  - the per-head V bias is attention-invariant (attn rows sum to 1), so it is
    folded into a host-side constant: bf_eff = bf + bv_flat @ Wf.
"""

import os
import sys

for _p in ("/opt/trn_rl_repo", "/root/.axon_site/_ro/trn_rl_repo"):
    if os.path.isdir(_p) and _p not in sys.path:
        sys.path.insert(0, _p)

import numpy as np
import ml_dtypes

import concourse.bass as bass
import concourse.mybir as mybir
import concourse.tile as tile
from concourse.bass_utils import run_bass_kernel_spmd

BF16 = mybir.dt.bfloat16
F32 = mybir.dt.float32

B, S, E, H = 4, 2048, 1024, 16
D = E // H
NCORES = 8
HPC = H // NCORES  # heads per core = 2
P = 128
CPC = HPC * D  # concat dims per core = 128


TC = tile.TileContext


def split_ctrl_waits(nc):
    """This walrus build rejects instructions carrying more than one
    sem-wait (setupSyncWait: 'Too many sync wait commands'); hoist extra
    waits into single-wait NoOps placed just before the offending
    instruction (same engine => same sequencer order => same semantics)."""
    for f in nc.m.functions:
        for blk in f.blocks:
            insts = blk.instructions
            if not any(
                i.sync_info is not None and len(i.sync_info.on_wait) > 1
                for i in insts
            ):
                continue
            out = []
            for inst in insts:
                si = inst.sync_info
                cap = 1  # this walrus build allows ONE wait per instruction
                if si is not None and len(si.on_wait) > cap:
                    waits = list(si.on_wait)
                    si.on_wait = waits[:cap]
                    for j, w in enumerate(waits[cap:]):
                        out.append(
                            mybir.InstNoOp(
                                name=f"{inst.name}_sw{j}",
                                engine=inst.engine,
                                sync_info=mybir.SyncInfo(
                                    on_wait=[w], on_update=[]
                                ),
                                bass_nofuse=True,
                            )
                        )
                out.append(inst)
            blk.instructions = out


def build_program(nb=B, seq=S, rep_loop=True):
    """Build the per-core Bass program. nb/seq shrinkable for simulation."""
    ET = E // P          # E tiles (contraction tiles for projections)
    TT = seq // P        # t tiles per batch
    SBLK = min(512, seq)  # s-block (matmul moving free dim)
    NSB = seq // SBLK    # s blocks per batch
    SM = SBLK // P       # 128-token slices per s-block

    nc = bass.Bass()
    xT = nc.dram_tensor("xT", [nb, E, seq], BF16, kind="ExternalInput")
    wq = nc.dram_tensor("wq", [E, CPC], BF16, kind="ExternalInput")
    wk = nc.dram_tensor("wk", [E, CPC], BF16, kind="ExternalInput")
    wv = nc.dram_tensor("wv", [E, CPC], BF16, kind="ExternalInput")
    bq = nc.dram_tensor("bq", [CPC, 1], F32, kind="ExternalInput")
    bk = nc.dram_tensor("bk", [CPC, 1], F32, kind="ExternalInput")
    wf = nc.dram_tensor("wf", [CPC, E], BF16, kind="ExternalInput")
    if rep_loop:
        reps = nc.dram_tensor(
            "reps", [1, 1], mybir.dt.int32, kind="ExternalInput")
    out = nc.dram_tensor("out", [nb * seq, E], F32, kind="ExternalOutput")

    scale = 1.0 / np.sqrt(D)

    with TC(nc) as tc:
        with (
            tc.tile_pool(name="consts", bufs=1) as consts,
            tc.tile_pool(name="xpool", bufs=2) as xpool,
            tc.tile_pool(name="qkpool", bufs=2) as qkpool,
            tc.tile_pool(name="vpool", bufs=2) as vpool,
            tc.tile_pool(name="ptpool", bufs=2) as ptpool,
            tc.tile_pool(name="cctpool", bufs=2) as cctpool,
            tc.tile_pool(name="normpool", bufs=2) as normpool,
            tc.tile_pool(name="osbpool", bufs=3) as osbpool,
            tc.tile_pool(name="ps_sc", bufs=2, space="PSUM") as ps_sc,
            tc.tile_pool(name="ps_ot", bufs=1, space="PSUM") as ps_ot,
            tc.tile_pool(name="ps_mm", bufs=2, space="PSUM") as ps_mm,
        ):
            # ---- constants / weights (loaded once) ----
            wq_sb = consts.tile([P, ET, CPC], BF16, tag="wq")
            wk_sb = consts.tile([P, ET, CPC], BF16, tag="wk")
            wv_sb = consts.tile([P, ET, CPC], BF16, tag="wv")
            nc.sync.dma_start(wq_sb[:], wq.rearrange("(eo ei) d -> ei eo d", ei=P))
            nc.sync.dma_start(wk_sb[:], wk.rearrange("(eo ei) d -> ei eo d", ei=P))
            nc.sync.dma_start(wv_sb[:], wv.rearrange("(eo ei) d -> ei eo d", ei=P))
            bq_sb = consts.tile([CPC, 1], F32, tag="bq")
            bk_sb = consts.tile([CPC, 1], F32, tag="bk")
            nc.sync.dma_start(bq_sb[:], bq[:])
            nc.sync.dma_start(bk_sb[:], bk[:])
            wf_sb = consts.tile([CPC, E], BF16, tag="wf")
            nc.sync.dma_start(wf_sb[:], wf[:])
            ones_sb = consts.tile([1, D], F32, tag="ones")
            nc.vector.memset(ones_sb[:], 1.0)

            # Runtime repetition count (for timing loops; 1 in production).
            if rep_loop:
                rt = consts.tile([1, 1], mybir.dt.int32, tag="rt")
                nc.sync.dma_start(rt[:], reps[:])
                rregs = []
                for e in mybir.ALL_ENGINES:
                    eng = nc.engines[e]
                    rg = eng.alloc_register(f"reps_{e.name}")
                    eng.reg_load(rg, rt[0:1, 0:1])
                    rregs.append(rg)
                rval = nc.snap(
                    bass.RegisterHandles(rregs), min_val=1, max_val=256)
                loop_cm = tc.For_i(
                    0, rval, 1, hint_engines=tuple(mybir.ALL_ENGINES))
                loop_cm.__enter__()
            for b in range(nb):
                # ---- load xT for this batch: [E, seq] -> [128, ET, seq] ----
                # (per-E-tile DMAs so the first projection matmuls can start
                # before the whole batch has landed)
                xb = xpool.tile([P, ET, seq], BF16, tag="xb")
                for e in range(ET):
                    nc.sync.dma_start(
                        xb[:, e, :], xT[b, e * P:(e + 1) * P, :]
                    )

                # ---- Q^T / K^T projections: [CPC, seq] (d on partitions) ----
                qt = qkpool.tile([CPC, seq], BF16, tag="qt")
                kt = qkpool.tile([CPC, seq], BF16, tag="kt")
                for w_sb, b_sb, dst in ((wq_sb, bq_sb, qt), (wk_sb, bk_sb, kt)):
                    for sb in range(NSB):
                        ps = ps_mm.tile([P, SBLK], F32, tag="mm")
                        for e in range(ET):
                            nc.tensor.matmul(
                                ps[:],
                                w_sb[:, e, :],
                                xb[:, e, sb * SBLK:(sb + 1) * SBLK],
                                start=(e == 0),
                                stop=(e == ET - 1),
                            )
                        nc.vector.tensor_scalar(
                            dst[:, sb * SBLK:(sb + 1) * SBLK],
                            ps[:],
                            b_sb[:, 0:1],
                            None,
                            mybir.AluOpType.add,
                        )

                # ---- V projection, natural [t, d] layout + ones column ----
                # vt[:, 2t+h, 0:64] = V for head h, t-tile t; vt[:, *, 64] = 1
                vt = vpool.tile([P, 2 * TT, D + 1], BF16, tag="vt")
                nc.vector.memset(vt[:, :, D:D + 1], 1.0)
                for t in range(TT):
                    psv = ps_mm.tile([P, CPC], F32, tag="mm")
                    for e in range(ET):
                        nc.tensor.matmul(
                            psv[:],
                            xb[:, e, t * P:(t + 1) * P],
                            wv_sb[:, e, :],
                            start=(e == 0),
                            stop=(e == ET - 1),
                        )
                    for h in range(HPC):
                        nc.vector.tensor_copy(
                            out=vt[:, 2 * t + h, 0:D],
                            in_=psv[:, h * D:(h + 1) * D],
                        )

                # ---- attention per s-block ----
                for sb in range(NSB):
                    ssl = slice(sb * SBLK, (sb + 1) * SBLK)
                    ots = []
                    for h in range(HPC):
                        ot_tile = ps_ot.tile(
                            [D + 1, SBLK], F32, tag=f"ot{h}", name=f"ot{h}"
                        )
                        ots.append(ot_tile)
                    pt_prev = None
                    for t in range(TT):
                        # both heads' scores side by side in one 2-bank PSUM
                        # tile -> a single exp op per t-step (ACT op overhead
                        # is 172 cycles, so fewer+bigger ops matter)
                        sc = ps_sc.tile([P, HPC * SBLK], F32, tag="sc")
                        for h in range(HPC):
                            nc.tensor.matmul(
                                sc[:, h * SBLK:(h + 1) * SBLK],
                                kt[h * D:(h + 1) * D, t * P:(t + 1) * P],
                                qt[h * D:(h + 1) * D, ssl],
                                start=True,
                                stop=True,
                            )
                        pt = ptpool.tile([P, HPC * SBLK], BF16, tag="pt")
                        nc.scalar.activation(
                            pt[:], sc[:],
                            mybir.ActivationFunctionType.Exp,
                            scale=float(scale),
                        )
                        # AV lags one t-step so PE stays busy while ACT exps
                        if t > 0:
                            for h in range(HPC):
                                nc.tensor.matmul(
                                    ots[h][:],
                                    vt[:, 2 * (t - 1) + h, :],
                                    pt_prev[:, h * SBLK:(h + 1) * SBLK],
                                    start=(t - 1 == 0),
                                    stop=False,
                                )
                        pt_prev = pt
                    for h in range(HPC):
                        nc.tensor.matmul(
                            ots[h][:],
                            vt[:, 2 * (TT - 1) + h, :],
                            pt_prev[:, h * SBLK:(h + 1) * SBLK],
                            start=(TT == 1),
                            stop=True,
                        )

                    # ---- normalize + concat: cct[h*64+j, s] ----
                    cct = cctpool.tile([CPC, SBLK], BF16, tag="cct")
                    for h in range(HPC):
                        recip = normpool.tile([1, SBLK], F32, tag="recip")
                        nc.vector.reciprocal(recip[:], ots[h][D:D + 1, :])
                        bc = ps_mm.tile([D, SBLK], F32, tag="mm")
                        nc.tensor.matmul(
                            bc[:], ones_sb[:], recip[:], start=True, stop=True
                        )
                        bcs = normpool.tile([D, SBLK], F32, tag="bcs")
                        nc.vector.tensor_copy(out=bcs[:], in_=bc[:])
                        nc.vector.tensor_mul(
                            out=cct[h * D:(h + 1) * D, :],
                            in0=ots[h][0:D, :],
                            in1=bcs[:],
                        )

                    # ---- output projection (core's 128-row slice of Wf) ----
                    for m in range(SM):
                        osb = osbpool.tile([P, E], F32, tag="osb")
                        for n in range(E // 512):
                            pso = ps_mm.tile([P, 512], F32, tag="mm")
                            nc.tensor.matmul(
                                pso[:],
                                cct[:, m * P:(m + 1) * P],
                                wf_sb[:, n * 512:(n + 1) * 512],
                                start=True,
                                stop=True,
                            )
                            nc.vector.tensor_copy(
                                out=osb[:, n * 512:(n + 1) * 512], in_=pso[:]
                            )
                        row = b * seq + sb * SBLK + m * P
                        nc.sync.dma_start(out[row:row + P, :], osb[:])
            if rep_loop:
                loop_cm.__exit__(None, None, None)
    return nc


def _prep_inputs(x, Wq, bq, Wk, bk, Wv, Wf):
    """Host-side slicing/packing. Returns per-core input maps."""
    bf16 = ml_dtypes.bfloat16
    xT = np.ascontiguousarray(np.transpose(x, (0, 2, 1))).astype(bf16)
    in_maps = []
    for c in range(NCORES):
        h0, h1 = HPC * c, HPC * c + HPC
        in_maps.append({
            "xT": xT,
            "wq": np.ascontiguousarray(
                np.concatenate(list(Wq[h0:h1]), axis=1)).astype(bf16),
            "wk": np.ascontiguousarray(
                np.concatenate(list(Wk[h0:h1]), axis=1)).astype(bf16),
            "wv": np.ascontiguousarray(
                np.concatenate(list(Wv[h0:h1]), axis=1)).astype(bf16),
            "bq": np.ascontiguousarray(
                bq[h0:h1].reshape(CPC, 1)).astype(np.float32),
            "bk": np.ascontiguousarray(
                bk[h0:h1].reshape(CPC, 1)).astype(np.float32),
            "wf": np.ascontiguousarray(
                Wf[c * CPC:(c + 1) * CPC]).astype(bf16),
        })
    return in_maps


def run(x, Wq, bq, Wk, bk, Wv, bv, Wf, bf):
    """Run on 8 cores; returns (output [B,S,E] f32, BassKernelResults)."""
    x = np.asarray(x, dtype=np.float32)
    in_maps = _prep_inputs(
        x, np.asarray(Wq), np.asarray(bq), np.asarray(Wk), np.asarray(bk),
        np.asarray(Wv), np.asarray(Wf))
    for m in in_maps:
        m["reps"] = np.array([[1]], dtype=np.int32)
    nc = build_program()
    split_ctrl_waits(nc)
    res = run_bass_kernel_spmd(nc, in_maps, list(range(NCORES)))
    acc = np.zeros((B * S, E), dtype=np.float32)
    for c in range(NCORES):
        acc += res.results[c]["out"]
    bf_eff = (np.asarray(bf, dtype=np.float32)
              + np.asarray(bv, dtype=np.float32).reshape(-1)
              @ np.asarray(Wf, dtype=np.float32))
    acc += bf_eff[None, :]
    return acc.reshape(B, S, E), res


def kernel(**inputs):
    out, _ = run(**inputs)
    return out


# revision 21
# speedup vs baseline: 1.3552x; 1.3552x over previous
"""Multi-head attention TRN2 kernel.

Problem: B=4, S=2048, E=1024, H=16 heads of D=64.
Sharding: tensor parallelism over heads — each of the 8 cores computes 2 heads
end-to-end (QKV projections, attention, its slice of the output projection) and
returns a partial [B*S, E] output; the host sums the 8 partials and adds the
combined bias constant.

Per-core dataflow (all matmuls bf16 inputs, fp32 PSUM accumulation):
  - host passes x pre-transposed (xT: [B, E, S]) so Q^T/K^T come straight out
    of the PE in [d, s] layout and V in natural [t, d] layout — no on-chip
    transposes anywhere.
  - scores are computed transposed (S^T: [t, s]) so softmax's reduction (over
    t) can be fused into the attention*V matmul: V gets an appended column of
    ones, so row 64 of the AV output is the softmax denominator.
  - exp runs on the scalar engine with the 1/sqrt(D) scale folded in; scores
    are bounded (|s|<3) so no max-subtraction pass is needed.
  - normalization multiplies the AV output by 1/denominator broadcast across
    partitions via a K=1 matmul (ones ⊗ recip).
  - the per-head V bias is attention-invariant (attn rows sum to 1), so it is
    folded into a host-side constant: bf_eff = bf + bv_flat @ Wf.
"""

import os
import sys

for _p in ("/opt/trn_rl_repo", "/root/.axon_site/_ro/trn_rl_repo"):
    if os.path.isdir(_p) and _p not in sys.path:
        sys.path.insert(0, _p)

import numpy as np
import ml_dtypes

import concourse.bass as bass
import concourse.mybir as mybir
import concourse.tile as tile
from concourse.bass_utils import run_bass_kernel_spmd

BF16 = mybir.dt.bfloat16
F32 = mybir.dt.float32

B, S, E, H = 4, 2048, 1024, 16
D = E // H
NCORES = 8
HPC = H // NCORES  # heads per core = 2
P = 128
CPC = HPC * D  # concat dims per core = 128


TC = tile.TileContext


def split_ctrl_waits(nc):
    """This walrus build rejects instructions carrying more than one
    sem-wait (setupSyncWait: 'Too many sync wait commands'); hoist extra
    waits into single-wait NoOps placed just before the offending
    instruction (same engine => same sequencer order => same semantics)."""
    for f in nc.m.functions:
        for blk in f.blocks:
            insts = blk.instructions
            if not any(
                i.sync_info is not None and len(i.sync_info.on_wait) > 1
                for i in insts
            ):
                continue
            out = []
            for inst in insts:
                si = inst.sync_info
                cap = 1  # this walrus build allows ONE wait per instruction
                if si is not None and len(si.on_wait) > cap:
                    waits = list(si.on_wait)
                    si.on_wait = waits[:cap]
                    for j, w in enumerate(waits[cap:]):
                        out.append(
                            mybir.InstNoOp(
                                name=f"{inst.name}_sw{j}",
                                engine=inst.engine,
                                sync_info=mybir.SyncInfo(
                                    on_wait=[w], on_update=[]
                                ),
                                bass_nofuse=True,
                            )
                        )
                out.append(inst)
            blk.instructions = out


def build_program(nb=B, seq=S, rep_loop=True):
    """Build the per-core Bass program. nb/seq shrinkable for simulation."""
    ET = E // P          # E tiles (contraction tiles for projections)
    TT = seq // P        # t tiles per batch
    SBLK = min(512, seq)  # s-block (matmul moving free dim)
    NSB = seq // SBLK    # s blocks per batch
    SM = SBLK // P       # 128-token slices per s-block

    nc = bass.Bass()
    xT = nc.dram_tensor("xT", [nb, E, seq], BF16, kind="ExternalInput")
    wq = nc.dram_tensor("wq", [E, CPC], BF16, kind="ExternalInput")
    wk = nc.dram_tensor("wk", [E, CPC], BF16, kind="ExternalInput")
    wv = nc.dram_tensor("wv", [E, CPC], BF16, kind="ExternalInput")
    bq = nc.dram_tensor("bq", [CPC, 1], F32, kind="ExternalInput")
    bk = nc.dram_tensor("bk", [CPC, 1], F32, kind="ExternalInput")
    wf = nc.dram_tensor("wf", [CPC, E], BF16, kind="ExternalInput")
    if rep_loop:
        reps = nc.dram_tensor(
            "reps", [1, 1], mybir.dt.int32, kind="ExternalInput")
    out = nc.dram_tensor("out", [nb * seq, E], BF16, kind="ExternalOutput")

    scale = 1.0 / np.sqrt(D)

    with TC(nc) as tc:
        with (
            tc.tile_pool(name="consts", bufs=1) as consts,
            tc.tile_pool(name="xpool", bufs=2) as xpool,
            tc.tile_pool(name="qkpool", bufs=2) as qkpool,
            tc.tile_pool(name="vpool", bufs=2) as vpool,
            tc.tile_pool(name="ptpool", bufs=2) as ptpool,
            tc.tile_pool(name="cctpool", bufs=2) as cctpool,
            tc.tile_pool(name="normpool", bufs=2) as normpool,
            tc.tile_pool(name="osbpool", bufs=3) as osbpool,
            tc.tile_pool(name="ps_sc", bufs=2, space="PSUM") as ps_sc,
            tc.tile_pool(name="ps_ot", bufs=1, space="PSUM") as ps_ot,
            tc.tile_pool(name="ps_mm", bufs=2, space="PSUM") as ps_mm,
        ):
            # ---- constants / weights (loaded once) ----
            wq_sb = consts.tile([P, ET, CPC], BF16, tag="wq")
            wk_sb = consts.tile([P, ET, CPC], BF16, tag="wk")
            wv_sb = consts.tile([P, ET, CPC], BF16, tag="wv")
            nc.sync.dma_start(wq_sb[:], wq.rearrange("(eo ei) d -> ei eo d", ei=P))
            nc.sync.dma_start(wk_sb[:], wk.rearrange("(eo ei) d -> ei eo d", ei=P))
            nc.sync.dma_start(wv_sb[:], wv.rearrange("(eo ei) d -> ei eo d", ei=P))
            bq_sb = consts.tile([CPC, 1], F32, tag="bq")
            bk_sb = consts.tile([CPC, 1], F32, tag="bk")
            nc.sync.dma_start(bq_sb[:], bq[:])
            nc.sync.dma_start(bk_sb[:], bk[:])
            wf_sb = consts.tile([CPC, E], BF16, tag="wf")
            nc.sync.dma_start(wf_sb[:], wf[:])
            ones_sb = consts.tile([1, D], F32, tag="ones")
            nc.vector.memset(ones_sb[:], 1.0)

            # Runtime repetition count (for timing loops; 1 in production).
            if rep_loop:
                rt = consts.tile([1, 1], mybir.dt.int32, tag="rt")
                nc.sync.dma_start(rt[:], reps[:])
                rregs = []
                for e in mybir.ALL_ENGINES:
                    eng = nc.engines[e]
                    rg = eng.alloc_register(f"reps_{e.name}")
                    eng.reg_load(rg, rt[0:1, 0:1])
                    rregs.append(rg)
                rval = nc.snap(
                    bass.RegisterHandles(rregs), min_val=1, max_val=256)
                loop_cm = tc.For_i(
                    0, rval, 1, hint_engines=tuple(mybir.ALL_ENGINES))
                loop_cm.__enter__()
            for b in range(nb):
                # ---- load xT for this batch: [E, seq] -> [128, ET, seq] ----
                # (per-E-tile DMAs so the first projection matmuls can start
                # before the whole batch has landed)
                xb = xpool.tile([P, ET, seq], BF16, tag="xb")
                for e in range(ET):
                    nc.sync.dma_start(
                        xb[:, e, :], xT[b, e * P:(e + 1) * P, :]
                    )

                # ---- Q^T / K^T projections: [CPC, seq] (d on partitions) ----
                qt = qkpool.tile([CPC, seq], BF16, tag="qt")
                kt = qkpool.tile([CPC, seq], BF16, tag="kt")
                for w_sb, b_sb, dst in ((wq_sb, bq_sb, qt), (wk_sb, bk_sb, kt)):
                    for sb in range(NSB):
                        ps = ps_mm.tile([P, SBLK], F32, tag="mm")
                        for e in range(ET):
                            nc.tensor.matmul(
                                ps[:],
                                w_sb[:, e, :],
                                xb[:, e, sb * SBLK:(sb + 1) * SBLK],
                                start=(e == 0),
                                stop=(e == ET - 1),
                            )
                        nc.vector.tensor_scalar(
                            dst[:, sb * SBLK:(sb + 1) * SBLK],
                            ps[:],
                            b_sb[:, 0:1],
                            None,
                            mybir.AluOpType.add,
                        )

                # ---- V projection, natural [t, d] layout + ones column ----
                # vt[:, 2t+h, 0:64] = V for head h, t-tile t; vt[:, *, 64] = 1
                vt = vpool.tile([P, 2 * TT, D + 1], BF16, tag="vt")
                nc.vector.memset(vt[:, :, D:D + 1], 1.0)
                for t in range(TT):
                    psv = ps_mm.tile([P, CPC], F32, tag="mm")
                    for e in range(ET):
                        nc.tensor.matmul(
                            psv[:],
                            xb[:, e, t * P:(t + 1) * P],
                            wv_sb[:, e, :],
                            start=(e == 0),
                            stop=(e == ET - 1),
                        )
                    for h in range(HPC):
                        nc.vector.tensor_copy(
                            out=vt[:, 2 * t + h, 0:D],
                            in_=psv[:, h * D:(h + 1) * D],
                        )

                # ---- attention per s-block ----
                for sb in range(NSB):
                    ssl = slice(sb * SBLK, (sb + 1) * SBLK)
                    ots = []
                    for h in range(HPC):
                        ot_tile = ps_ot.tile(
                            [D + 1, SBLK], F32, tag=f"ot{h}", name=f"ot{h}"
                        )
                        ots.append(ot_tile)
                    pt_prev = None
                    for t in range(TT):
                        # both heads' scores side by side in one 2-bank PSUM
                        # tile -> a single exp op per t-step (ACT op overhead
                        # is 172 cycles, so fewer+bigger ops matter)
                        sc = ps_sc.tile([P, HPC * SBLK], F32, tag="sc")
                        for h in range(HPC):
                            nc.tensor.matmul(
                                sc[:, h * SBLK:(h + 1) * SBLK],
                                kt[h * D:(h + 1) * D, t * P:(t + 1) * P],
                                qt[h * D:(h + 1) * D, ssl],
                                start=True,
                                stop=True,
                            )
                        pt = ptpool.tile([P, HPC * SBLK], BF16, tag="pt")
                        nc.scalar.activation(
                            pt[:], sc[:],
                            mybir.ActivationFunctionType.Exp,
                            scale=float(scale),
                        )
                        # AV lags one t-step so PE stays busy while ACT exps
                        if t > 0:
                            for h in range(HPC):
                                nc.tensor.matmul(
                                    ots[h][:],
                                    vt[:, 2 * (t - 1) + h, :],
                                    pt_prev[:, h * SBLK:(h + 1) * SBLK],
                                    start=(t - 1 == 0),
                                    stop=False,
                                )
                        pt_prev = pt
                    for h in range(HPC):
                        nc.tensor.matmul(
                            ots[h][:],
                            vt[:, 2 * (TT - 1) + h, :],
                            pt_prev[:, h * SBLK:(h + 1) * SBLK],
                            start=(TT == 1),
                            stop=True,
                        )

                    # ---- normalize + concat: cct[h*64+j, s] ----
                    cct = cctpool.tile([CPC, SBLK], BF16, tag="cct")
                    for h in range(HPC):
                        recip = normpool.tile([1, SBLK], F32, tag="recip")
                        nc.vector.reciprocal(recip[:], ots[h][D:D + 1, :])
                        bc = ps_mm.tile([D, SBLK], F32, tag="mm")
                        nc.tensor.matmul(
                            bc[:], ones_sb[:], recip[:], start=True, stop=True
                        )
                        bcs = normpool.tile([D, SBLK], F32, tag="bcs")
                        nc.vector.tensor_copy(out=bcs[:], in_=bc[:])
                        nc.vector.tensor_mul(
                            out=cct[h * D:(h + 1) * D, :],
                            in0=ots[h][0:D, :],
                            in1=bcs[:],
                        )

                    # ---- output projection (core's 128-row slice of Wf) ----
                    for m in range(SM):
                        osb = osbpool.tile([P, E], BF16, tag="osb")
                        for n in range(E // 512):
                            pso = ps_mm.tile([P, 512], F32, tag="mm")
                            nc.tensor.matmul(
                                pso[:],
                                cct[:, m * P:(m + 1) * P],
                                wf_sb[:, n * 512:(n + 1) * 512],
                                start=True,
                                stop=True,
                            )
                            nc.vector.tensor_copy(
                                out=osb[:, n * 512:(n + 1) * 512], in_=pso[:]
                            )
                        row = b * seq + sb * SBLK + m * P
                        nc.sync.dma_start(out[row:row + P, :], osb[:])
            if rep_loop:
                loop_cm.__exit__(None, None, None)
    return nc


def _prep_inputs(x, Wq, bq, Wk, bk, Wv, Wf):
    """Host-side slicing/packing. Returns per-core input maps."""
    bf16 = ml_dtypes.bfloat16
    xT = np.ascontiguousarray(np.transpose(x, (0, 2, 1))).astype(bf16)
    in_maps = []
    for c in range(NCORES):
        h0, h1 = HPC * c, HPC * c + HPC
        in_maps.append({
            "xT": xT,
            "wq": np.ascontiguousarray(
                np.concatenate(list(Wq[h0:h1]), axis=1)).astype(bf16),
            "wk": np.ascontiguousarray(
                np.concatenate(list(Wk[h0:h1]), axis=1)).astype(bf16),
            "wv": np.ascontiguousarray(
                np.concatenate(list(Wv[h0:h1]), axis=1)).astype(bf16),
            "bq": np.ascontiguousarray(
                bq[h0:h1].reshape(CPC, 1)).astype(np.float32),
            "bk": np.ascontiguousarray(
                bk[h0:h1].reshape(CPC, 1)).astype(np.float32),
            "wf": np.ascontiguousarray(
                Wf[c * CPC:(c + 1) * CPC]).astype(bf16),
        })
    return in_maps


def run(x, Wq, bq, Wk, bk, Wv, bv, Wf, bf):
    """Run on 8 cores; returns (output [B,S,E] f32, BassKernelResults)."""
    x = np.asarray(x, dtype=np.float32)
    in_maps = _prep_inputs(
        x, np.asarray(Wq), np.asarray(bq), np.asarray(Wk), np.asarray(bk),
        np.asarray(Wv), np.asarray(Wf))
    for m in in_maps:
        m["reps"] = np.array([[1]], dtype=np.int32)
    nc = build_program()
    split_ctrl_waits(nc)
    res = run_bass_kernel_spmd(nc, in_maps, list(range(NCORES)))
    acc = np.zeros((B * S, E), dtype=np.float32)
    for c in range(NCORES):
        acc += res.results[c]["out"].astype(np.float32)
    bf_eff = (np.asarray(bf, dtype=np.float32)
              + np.asarray(bv, dtype=np.float32).reshape(-1)
              @ np.asarray(Wf, dtype=np.float32))
    acc += bf_eff[None, :]
    return acc.reshape(B, S, E), res


def kernel(**inputs):
    out, _ = run(**inputs)
    return out
